# revision 28
# baseline (speedup 1.0000x reference)
"""Sparse-attention kernel for Trainium2 (8 NeuronCores, data-parallel).

reference (per batch b):
    h     = relu(k @ w1 + q @ w2 + bias)          [L, D]
    alpha = h @ w0.T                               [L, S]
    alpha = where(mask == 0, -1e9, alpha)
    alpha = softmax(alpha, axis=L)
    out   = alpha.T @ v                            [S, D]

Device mapping (per core, 512 batches):
  - host pre-transposes q,k to [D, Bc*L] so the D-contraction has D on
    partitions; weights are replicated.
  - softmax over L has no max-subtraction (alpha is O(3), masked lanes
    multiply by 0 after exp — identical to exp(-1e9)=0 in the reference).
  - v is augmented with a ones column so the step-4 matmul produces the
    softmax denominator per output partition; the evacuation op divides.
  - everything on the wire is bf16 except the mask, which is fp8_e4m3 in
    DRAM (0/1 exact) and cast to bf16 by the SWDGE during the load; the
    output is stored bf16 and widened to f32 on the host.  h is bf16 so
    the alpha matmuls run at 1 cycle/row (f32r with a 50-wide moving dim
    is 4 cycles/row on TRN2).
  - DMA queues: qt -> sync (HWDGE), kt -> scalar (HWDGE) — both 128-
    partition, spreading over all 16 SDMA engines; va/mk/out (100-
    partition) -> gpsimd (SWDGE), which round-robins descriptors over
    all 16 engines regardless of partition count.  Loads are merged over
    super-groups of 4 groups (~820 KB per transfer).
  - emission is phase-major over super-groups of 4; step 4 (+ the output
    store) trails by one super-group and is emitted FIRST in each
    iteration so its Act/DVE divides sit early in those queues and the
    gpsimd sequencer isn't stalled waiting on them.
"""
import os
import sys

for p in ("/opt/trn_rl_repo", "/root/.axon_site", "/root/.axon_site/_ro/trn_rl_repo"):
    if os.path.isdir(p) and p not in sys.path:
        sys.path.append(p)

import numpy as np
import ml_dtypes

import concourse.bass as bass
import concourse.tile as tile
from concourse import mybir
from concourse.bass_utils import run_bass_kernel_spmd

# ---------------------------------------------------------------------------
# Workaround for this walrus build's limit of ONE sync-wait per instruction:
# hoist extra waits onto same-engine NoOps inserted just before.
_wsplit_counter = [0]


def _split_multi_waits(nc):
    for fn in nc.m.functions:
        for bb in fn.blocks:
            out = []
            changed = False
            for inst in bb.instructions:
                si = inst.sync_info
                if si is not None and len(si.on_wait) > 1:
                    waits = list(si.on_wait)
                    for w in waits[:-1]:
                        _wsplit_counter[0] += 1
                        nop = mybir.InstNoOp(
                            name=f"I-wsplit-{_wsplit_counter[0]}",
                            ins=[],
                            outs=[],
                            engine=inst.engine,
                        )
                        nop.sync_info = mybir.SyncInfo(on_wait=[w], on_update=[])
                        out.append(nop)
                    inst.sync_info = mybir.SyncInfo(
                        on_wait=[waits[-1]], on_update=list(si.on_update)
                    )
                    changed = True
                out.append(inst)
            if changed:
                bb.instructions = out


# ---------------------------------------------------------------------------
B, L, D, S = 4096, 50, 256, 50
M = 8                 # cores
Bc = B // M           # batches per core
G = 8                 # batches per group
NG = Bc // G          # groups per core (64)
TOK = G * L           # tokens per group (400)
SG = 4                # groups per super-group
NSG = NG // SG        # super-groups per core (16)
VA_W = D + 2          # v augmented with [1, 0] columns (even free dim)

f32 = mybir.dt.float32
bf16 = mybir.dt.bfloat16
fp8 = mybir.dt.float8e4
AF = mybir.ActivationFunctionType

_cache = {}


def _build():
    if "nc" in _cache:
        return _cache["nc"]
    nc = bass.Bass("TRN2", target_bir_lowering=False, debug=False)
    # group-major DRAM layouts: every transfer reads/writes one contiguous
    # block (~0.4-0.8 MB), so each SDMA engine streams sequential addresses
    # instead of 1.6-8 KB rows scattered at ~100 KB strides
    qt_d = nc.dram_tensor("qt", [NG, 128, 2, TOK], bf16, kind="ExternalInput").ap()
    kt_d = nc.dram_tensor("kt", [NG, 128, 2, TOK], bf16, kind="ExternalInput").ap()
    va_d = nc.dram_tensor("va", [NG, 100, 4, VA_W], bf16, kind="ExternalInput").ap()
    mk_d = nc.dram_tensor("mk", [NSG, 100, SG, 4, 2 * S], fp8, kind="ExternalInput").ap()
    w1_d = nc.dram_tensor("w1", [D, D], bf16, kind="ExternalInput").ap()
    w2_d = nc.dram_tensor("w2", [D, D], bf16, kind="ExternalInput").ap()
    w0t_d = nc.dram_tensor("w0t", [D, S], bf16, kind="ExternalInput").ap()
    bias_d = nc.dram_tensor("bias", [D, 1], f32, kind="ExternalInput").ap()
    out_d = nc.dram_tensor("out", [NG, 100, 4, D], bf16, kind="ExternalOutput").ap()

    with tile.TileContext(nc) as tc:
        with (
            tc.tile_pool(name="singles", bufs=1) as singles,
            tc.tile_pool(name="qk", bufs=16) as qk,
            tc.tile_pool(name="htp", bufs=5) as htp,
            tc.tile_pool(name="va", bufs=3) as vap,
            tc.tile_pool(name="mk", bufs=3) as mkp,
            tc.tile_pool(name="e0", bufs=4) as e0p,
            tc.tile_pool(name="e", bufs=10) as epool,
            tc.tile_pool(name="rc", bufs=16) as rcp,
            tc.tile_pool(name="osb", bufs=6) as osb,
            tc.tile_pool(name="ht_ps", bufs=2, space="PSUM") as ht_ps,
            tc.tile_pool(name="al_ps", bufs=2, space="PSUM") as al_ps,
            tc.tile_pool(name="o_ps", bufs=4, space="PSUM") as o_ps,
        ):
            w1_t = singles.tile([128, 2, D], bf16)
            nc.sync.dma_start(w1_t[:], w1_d.rearrange("(c p) n -> p c n", p=128))
            w2_t = singles.tile([128, 2, D], bf16)
            nc.sync.dma_start(w2_t[:], w2_d.rearrange("(c p) n -> p c n", p=128))

            def emit_loads(s):
                # Per-group qt/kt transfers (~400 KB, HWDGE on sync):
                # fine-grained completion so step1 never waits on more data
                # than it needs.  va/mk merged per super-group on gpsimd
                # (SWDGE) to keep the Q7 descriptor-generation off the
                # critical supply path.
                qts, kts, vas = [], [], []
                for gi in range(SG):
                    g = s * SG + gi
                    qt_t = qk.tile([128, 2, TOK], bf16, tag="qt")
                    nc.sync.dma_start(qt_t[:], qt_d[g])
                    kt_t = qk.tile([128, 2, TOK], bf16, tag="kt")
                    nc.sync.dma_start(kt_t[:], kt_d[g])
                    va_t = vap.tile([100, 4, VA_W], bf16, tag="va")
                    nc.gpsimd.dma_start(va_t[:], va_d[g])
                    qts.append(qt_t); kts.append(kt_t); vas.append(va_t)
                mk_t = mkp.tile([100, SG, 4, 2 * S], bf16, tag="mk")
                nc.gpsimd.dma_start(mk_t[:], mk_d[s])
                return qts, kts, vas, mk_t

            def emit_s1(qt_t, kt_t):
                # step 1: ht = relu(w1.T @ kt + w2.T @ qt + bias)  [D, TOK]
                ht_t = htp.tile([128, 2, TOK], bf16)
                for co in range(2):
                    hp = ht_ps.tile([128, TOK], f32)
                    cs = slice(co * 128, (co + 1) * 128)
                    nc.tensor.matmul(hp[:], w1_t[:, 0, cs], kt_t[:, 0, :], start=True, stop=False)
                    nc.tensor.matmul(hp[:], w1_t[:, 1, cs], kt_t[:, 1, :], start=False, stop=False)
                    nc.tensor.matmul(hp[:], w2_t[:, 0, cs], qt_t[:, 0, :], start=False, stop=False)
                    nc.tensor.matmul(hp[:], w2_t[:, 1, cs], qt_t[:, 1, :], start=False, stop=True)
                    if co == 0:
                        nc.scalar.activation(
                            ht_t[:, co, :], hp[:], AF.Relu, bias=b_t[:, co, :]
                        )
                    else:
                        nc.vector.tensor_scalar(
                            ht_t[:, co, :], hp[:], b_t[:, co, :], 0.0,
                            mybir.AluOpType.add, mybir.AluOpType.max,
                        )
                return ht_t

            def emit_s3(ht_t, mk_t, gi):
                # step 2: alpha = ht.T @ w0t, one M=100 matmul pair per
                # token-pair (100 contiguous tokens)
                ap_t = al_ps.tile([100, 4, S], f32)
                for p in range(4):
                    bc = slice(p * 100, (p + 1) * 100)
                    nc.tensor.matmul(
                        ap_t[:, p, :], ht_t[:, 0, bc], w0_t[:, 0, :],
                        start=True, stop=False,
                    )
                    nc.tensor.matmul(
                        ap_t[:, p, :], ht_t[:, 1, bc], w0_t[:, 1, :],
                        start=False, stop=True,
                    )

                # step 3: e = exp(alpha) * mask, packed block-diagonally per
                # pair (even batch rows 0:50 x cols 0:50, odd batch rows
                # 50:100 x cols 50:100, zeros elsewhere) so one K=100 matmul
                # computes both batches of a pair.
                e0_t = e0p.tile([100, 4, S], bf16, tag="e0")
                nc.scalar.activation(e0_t[:], ap_t[:], AF.Exp)
                # One multiply builds the whole block-diagonal stationary:
                # the mask tensor is host-prepared at double width with the
                # off-diagonal blocks zeroed; e0 is read twice per row via a
                # stride-0 AP.
                e_t = epool.tile([100, 4, 2 * S], bf16, tag="e")
                e0_ap = e0_t[:]
                e0_bcast = bass.AP(
                    tensor=e0_ap.tensor,
                    offset=e0_ap.offset,
                    ap=[e0_ap.ap[0], e0_ap.ap[1], [0, 2], e0_ap.ap[2]],
                )
                # on gpsimd: all-SBUF operands, keeps the DVE queue free for
                # the PSUM evacuations and divides that gate PE progress
                nc.gpsimd.tensor_mul(
                    e_t[:].rearrange("l p (r s) -> l p r s", r=2),
                    e0_bcast,
                    mk_t[:, gi].rearrange("l p (r s) -> l p r s", r=2),
                )
                return e_t

            def emit_s5(e_t, va_t, o_t, gi):
                # step 4: [out | denom] = blockdiag(e).T @ [v | 1]
                for p in range(4):
                    op_t = o_ps.tile([100, VA_W], f32)
                    nc.tensor.matmul(
                        op_t[:], e_t[:, p, :], va_t[:, p, :],
                        start=True, stop=True,
                    )
                    rc_t = rcp.tile([100, 1], f32)
                    nc.vector.reciprocal(rc_t[:], op_t[:, D : D + 1])
                    if p % 2 == 0:
                        nc.scalar.activation(
                            o_t[:, p, :], op_t[:, 0:D], AF.Copy, scale=rc_t[:]
                        )
                    else:
                        nc.vector.tensor_scalar_mul(o_t[:, p, :], op_t[:, 0:D], rc_t[:])

            # Phase-major super-groups: batch each phase across SG groups
            # so the PE gets long dense matmul bursts and cross-engine
            # handoffs amortize.  Step 4 + the store trail by one
            # super-group and are emitted first so their Act/DVE divides
            # execute promptly and the output DMA issue doesn't stall the
            # gpsimd sequencer ahead of the next loads.
            # Queue-order design per iteration (engine programs execute in
            # emission order):
            #   loads(s+1)   — DMA issues a full iteration ahead of use;
            #                  qt+kt both on sync so the Act queue carries
            #                  no DMA work ahead of the divides
            #   s4(s-1,gi) interleaved with s1(s,gi) — the step1 matmuls
            #                  fill the PE while Act/DVE divides free the
            #                  o_ps PSUM bufs for the next step4 quad
            #   s2/s3(s)     — PE 32 MMs; exp/mult produce e for next iter
            loads = emit_loads(0)
            w0_t = singles.tile([128, 2, S], bf16)
            nc.sync.dma_start(w0_t[:], w0t_d.rearrange("(c p) s -> p c s", p=128))
            b_t = singles.tile([128, 2, 1], f32)
            nc.sync.dma_start(b_t[:], bias_d.rearrange("(c p) o -> p c o", p=128))
            def emit_s5_group(prev_s, e_t, va_t, gi):
                o_t = osb.tile([100, 4, D], bf16)
                emit_s5(e_t, va_t, o_t, gi)
                nc.gpsimd.dma_start(out_d[prev_s * SG + gi], o_t[:])

            prev = None
            for s in range(NSG):
                qts, kts, vas, mk_t = loads
                if s + 1 < NSG:
                    loads = emit_loads(s + 1)
                hts = []
                for gi in range(SG):
                    if prev is not None:
                        emit_s5_group(prev[0], prev[1][gi], prev[2][gi], gi)
                    hts.append(emit_s1(qts[gi], kts[gi]))
                e_ts = [emit_s3(hts[gi], mk_t, gi) for gi in range(SG)]
                prev = (s, e_ts, vas)
            for gi in range(SG):
                emit_s5_group(prev[0], prev[1][gi], prev[2][gi], gi)

    _split_multi_waits(nc)
    _cache["nc"] = nc
    return nc


def _make_in_maps(q, k, v, mask):
    in_maps = []
    for c in range(M):
        sl = slice(c * Bc, (c + 1) * Bc)
        # [NG, 128, 2, TOK]: each group's block contiguous in DRAM
        qs = np.ascontiguousarray(
            q[sl].reshape(NG, TOK, 2, 128).transpose(0, 3, 2, 1)
        ).astype(ml_dtypes.bfloat16)
        ks = np.ascontiguousarray(
            k[sl].reshape(NG, TOK, 2, 128).transpose(0, 3, 2, 1)
        ).astype(ml_dtypes.bfloat16)
        va = np.zeros((100, NG, 4, VA_W), dtype=ml_dtypes.bfloat16)
        va[:, :, :, :D] = v[sl].reshape(NG, 4, 100, D).transpose(2, 0, 1, 3).astype(ml_dtypes.bfloat16)
        va[:, :, :, D] = 1.0
        # [NG, 100, 4, VA_W]: group blocks contiguous
        va = np.ascontiguousarray(va.transpose(1, 0, 2, 3))
        m5 = mask[sl].reshape(NG, 4, 2, 50, S).transpose(3, 0, 1, 2, 4)
        mk = np.zeros((100, NG, 4, 2, S), dtype=ml_dtypes.float8_e4m3)
        mk[0:50, :, :, 0, :] = m5[:, :, :, 0, :].astype(ml_dtypes.float8_e4m3)
        mk[50:100, :, :, 1, :] = m5[:, :, :, 1, :].astype(ml_dtypes.float8_e4m3)
        mk = np.ascontiguousarray(
            mk.reshape(100, NSG, SG, 4, 2 * S).transpose(1, 0, 2, 3, 4)
        )
        in_maps.append({"qt": qs, "kt": ks, "va": va, "mk": mk})
    return in_maps


def _run(q, k, v, mask, attn_w0, attn_w1, attn_w2, attn_bias, **run_kwargs):
    nc = _build()
    w1 = np.ascontiguousarray(attn_w1).astype(ml_dtypes.bfloat16)
    w2 = np.ascontiguousarray(attn_w2).astype(ml_dtypes.bfloat16)
    w0t = np.ascontiguousarray(
        np.asarray(attn_w0, dtype=np.float32).T
    ).astype(ml_dtypes.bfloat16)
    bias = np.ascontiguousarray(
        np.asarray(attn_bias, dtype=np.float32).reshape(D, 1)
    )
    in_maps = _make_in_maps(
        np.asarray(q, dtype=np.float32),
        np.asarray(k, dtype=np.float32),
        np.asarray(v, dtype=np.float32),
        np.asarray(mask),
    )
    for im in in_maps:
        im.update({"w1": w1, "w2": w2, "w0t": w0t, "bias": bias})
    res = run_bass_kernel_spmd(nc, in_maps, core_ids=list(range(M)), **run_kwargs)
    # out: [NG, 100, 4, D] -> batch = g*8 + j*2 + (p>=50), s = p%50
    out = np.concatenate(
        [
            r["out"].astype(np.float32)
            .reshape(NG, 2, 50, 4, D)
            .transpose(0, 3, 1, 2, 4)
            .reshape(Bc, S, D)
            for r in res.results
        ],
        axis=0,
    )
    return out, res


def kernel(q, k, v, mask, attn_w0, attn_w1, attn_w2, attn_bias):
    out, _ = _run(q, k, v, mask, attn_w0, attn_w1, attn_w2, attn_bias)
    return out


# revision 29
# speedup vs baseline: 1.1968x; 1.1968x over previous
"""Sparse-attention kernel for Trainium2 (8 NeuronCores, data-parallel).

reference (per batch b):
    h     = relu(k @ w1 + q @ w2 + bias)          [L, D]
    alpha = h @ w0.T                               [L, S]
    alpha = where(mask == 0, -1e9, alpha)
    alpha = softmax(alpha, axis=L)
    out   = alpha.T @ v                            [S, D]

Device mapping (per core, 512 batches):
  - host pre-transposes q,k to [D, Bc*L] so the D-contraction has D on
    partitions; weights are replicated.
  - softmax over L has no max-subtraction (alpha is O(3), masked lanes
    multiply by 0 after exp — identical to exp(-1e9)=0 in the reference).
  - v is augmented with a ones column so the step-4 matmul produces the
    softmax denominator per output partition; the evacuation op divides.
  - everything on the wire is bf16 except the mask, which is fp8_e4m3 in
    DRAM (0/1 exact) and cast to bf16 by the SWDGE during the load; the
    output is stored bf16 and widened to f32 on the host.  h is bf16 so
    the alpha matmuls run at 1 cycle/row (f32r with a 50-wide moving dim
    is 4 cycles/row on TRN2).
  - DMA queues: qt -> sync (HWDGE), kt -> scalar (HWDGE) — both 128-
    partition, spreading over all 16 SDMA engines; va/mk/out (100-
    partition) -> gpsimd (SWDGE), which round-robins descriptors over
    all 16 engines regardless of partition count.  Loads are merged over
    super-groups of 4 groups (~820 KB per transfer).
  - emission is phase-major over super-groups of 4; step 4 (+ the output
    store) trails by one super-group and is emitted FIRST in each
    iteration so its Act/DVE divides sit early in those queues and the
    gpsimd sequencer isn't stalled waiting on them.
"""
import os
import sys

for p in ("/opt/trn_rl_repo", "/root/.axon_site", "/root/.axon_site/_ro/trn_rl_repo"):
    if os.path.isdir(p) and p not in sys.path:
        sys.path.append(p)

import numpy as np
import ml_dtypes

import concourse.bass as bass
import concourse.tile as tile
from concourse import mybir
from concourse.bass_utils import run_bass_kernel_spmd

# ---------------------------------------------------------------------------
# Workaround for this walrus build's limit of ONE sync-wait per instruction:
# hoist extra waits onto same-engine NoOps inserted just before.
_wsplit_counter = [0]


def _split_multi_waits(nc):
    for fn in nc.m.functions:
        for bb in fn.blocks:
            out = []
            changed = False
            for inst in bb.instructions:
                si = inst.sync_info
                if si is not None and len(si.on_wait) > 1:
                    waits = list(si.on_wait)
                    for w in waits[:-1]:
                        _wsplit_counter[0] += 1
                        nop = mybir.InstNoOp(
                            name=f"I-wsplit-{_wsplit_counter[0]}",
                            ins=[],
                            outs=[],
                            engine=inst.engine,
                        )
                        nop.sync_info = mybir.SyncInfo(on_wait=[w], on_update=[])
                        out.append(nop)
                    inst.sync_info = mybir.SyncInfo(
                        on_wait=[waits[-1]], on_update=list(si.on_update)
                    )
                    changed = True
                out.append(inst)
            if changed:
                bb.instructions = out


# ---------------------------------------------------------------------------
B, L, D, S = 4096, 50, 256, 50
M = 8                 # cores
Bc = B // M           # batches per core
G = 8                 # batches per group
NG = Bc // G          # groups per core (64)
TOK = G * L           # tokens per group (400)
SG = 4                # groups per super-group
NSG = NG // SG        # super-groups per core (16)
VA_W = D + 2          # v augmented with [1, 0] columns (even free dim)

f32 = mybir.dt.float32
bf16 = mybir.dt.bfloat16
fp8 = mybir.dt.float8e4
AF = mybir.ActivationFunctionType

_cache = {}


def _build():
    if "nc" in _cache:
        return _cache["nc"]
    nc = bass.Bass("TRN2", target_bir_lowering=False, debug=False)
    # group-major DRAM layouts: every transfer reads/writes one contiguous
    # block (~0.4-0.8 MB), so each SDMA engine streams sequential addresses
    # instead of 1.6-8 KB rows scattered at ~100 KB strides
    qt_d = nc.dram_tensor("qt", [NG, 128, 2, TOK], bf16, kind="ExternalInput").ap()
    kt_d = nc.dram_tensor("kt", [NG, 128, 2, TOK], bf16, kind="ExternalInput").ap()
    va_d = nc.dram_tensor("va", [100, NG, 4, VA_W], bf16, kind="ExternalInput").ap()
    mk_d = nc.dram_tensor("mk", [100, NSG, SG, 4, 2 * S], bf16, kind="ExternalInput").ap()
    w1_d = nc.dram_tensor("w1", [D, D], bf16, kind="ExternalInput").ap()
    w2_d = nc.dram_tensor("w2", [D, D], bf16, kind="ExternalInput").ap()
    w0t_d = nc.dram_tensor("w0t", [D, S], bf16, kind="ExternalInput").ap()
    bias_d = nc.dram_tensor("bias", [D, 1], f32, kind="ExternalInput").ap()
    out_d = nc.dram_tensor("out", [NG, 100, 4, D], bf16, kind="ExternalOutput").ap()

    with tile.TileContext(nc) as tc:
        with (
            tc.tile_pool(name="singles", bufs=1) as singles,
            tc.tile_pool(name="qk", bufs=16) as qk,
            tc.tile_pool(name="htp", bufs=5) as htp,
            tc.tile_pool(name="va", bufs=14) as vap,
            tc.tile_pool(name="mk", bufs=3) as mkp,
            tc.tile_pool(name="e0", bufs=4) as e0p,
            tc.tile_pool(name="e", bufs=10) as epool,
            tc.tile_pool(name="rc", bufs=16) as rcp,
            tc.tile_pool(name="osb", bufs=6) as osb,
            tc.tile_pool(name="ht_ps", bufs=2, space="PSUM") as ht_ps,
            tc.tile_pool(name="al_ps", bufs=2, space="PSUM") as al_ps,
            tc.tile_pool(name="o_ps", bufs=4, space="PSUM") as o_ps,
        ):
            w1_t = singles.tile([128, 2, D], bf16)
            nc.sync.dma_start(w1_t[:], w1_d.rearrange("(c p) n -> p c n", p=128))
            w2_t = singles.tile([128, 2, D], bf16)
            nc.sync.dma_start(w2_t[:], w2_d.rearrange("(c p) n -> p c n", p=128))

            def emit_loads(s):
                # Per-group qt/kt transfers (~400 KB, HWDGE on sync):
                # fine-grained completion so step1 never waits on more data
                # than it needs.  va/mk merged per super-group on gpsimd
                # (SWDGE) to keep the Q7 descriptor-generation off the
                # critical supply path.
                qts, kts, vas = [], [], []
                for gi in range(SG):
                    g = s * SG + gi
                    qt_t = qk.tile([128, 2, TOK], bf16, tag="qt")
                    nc.sync.dma_start(qt_t[:], qt_d[g])
                    kt_t = qk.tile([128, 2, TOK], bf16, tag="kt")
                    nc.sync.dma_start(kt_t[:], kt_d[g])
                    va_t = vap.tile([100, 4, VA_W], bf16, tag="va")
                    nc.gpsimd.dma_start(va_t[:], va_d[:, g])
                    qts.append(qt_t); kts.append(kt_t); vas.append(va_t)
                mk_t = mkp.tile([100, SG, 4, 2 * S], bf16, tag="mk")
                nc.gpsimd.dma_start(mk_t[:], mk_d[:, s])
                return qts, kts, vas, mk_t

            def emit_s1(qt_t, kt_t):
                # step 1: ht = relu(w1.T @ kt + w2.T @ qt + bias)  [D, TOK]
                ht_t = htp.tile([128, 2, TOK], bf16)
                for co in range(2):
                    hp = ht_ps.tile([128, TOK], f32)
                    cs = slice(co * 128, (co + 1) * 128)
                    nc.tensor.matmul(hp[:], w1_t[:, 0, cs], kt_t[:, 0, :], start=True, stop=False)
                    nc.tensor.matmul(hp[:], w1_t[:, 1, cs], kt_t[:, 1, :], start=False, stop=False)
                    nc.tensor.matmul(hp[:], w2_t[:, 0, cs], qt_t[:, 0, :], start=False, stop=False)
                    nc.tensor.matmul(hp[:], w2_t[:, 1, cs], qt_t[:, 1, :], start=False, stop=True)
                    if co == 0:
                        nc.scalar.activation(
                            ht_t[:, co, :], hp[:], AF.Relu, bias=b_t[:, co, :]
                        )
                    else:
                        nc.vector.tensor_scalar(
                            ht_t[:, co, :], hp[:], b_t[:, co, :], 0.0,
                            mybir.AluOpType.add, mybir.AluOpType.max,
                        )
                return ht_t

            def emit_s3(ht_t, mk_t, gi):
                # step 2: alpha = ht.T @ w0t, one M=100 matmul pair per
                # token-pair (100 contiguous tokens)
                ap_t = al_ps.tile([100, 4, S], f32)
                for p in range(4):
                    bc = slice(p * 100, (p + 1) * 100)
                    nc.tensor.matmul(
                        ap_t[:, p, :], ht_t[:, 0, bc], w0_t[:, 0, :],
                        start=True, stop=False,
                    )
                    nc.tensor.matmul(
                        ap_t[:, p, :], ht_t[:, 1, bc], w0_t[:, 1, :],
                        start=False, stop=True,
                    )

                # step 3: e = exp(alpha) * mask, packed block-diagonally per
                # pair (even batch rows 0:50 x cols 0:50, odd batch rows
                # 50:100 x cols 50:100, zeros elsewhere) so one K=100 matmul
                # computes both batches of a pair.
                e0_t = e0p.tile([100, 4, S], bf16, tag="e0")
                nc.scalar.activation(e0_t[:], ap_t[:], AF.Exp)
                # One multiply builds the whole block-diagonal stationary:
                # the mask tensor is host-prepared at double width with the
                # off-diagonal blocks zeroed; e0 is read twice per row via a
                # stride-0 AP.
                e_t = epool.tile([100, 4, 2 * S], bf16, tag="e")
                e0_ap = e0_t[:]
                e0_bcast = bass.AP(
                    tensor=e0_ap.tensor,
                    offset=e0_ap.offset,
                    ap=[e0_ap.ap[0], e0_ap.ap[1], [0, 2], e0_ap.ap[2]],
                )
                # on gpsimd: all-SBUF operands, keeps the DVE queue free for
                # the PSUM evacuations and divides that gate PE progress
                nc.vector.tensor_mul(
                    e_t[:].rearrange("l p (r s) -> l p r s", r=2),
                    e0_bcast,
                    mk_t[:, gi].rearrange("l p (r s) -> l p r s", r=2),
                )
                return e_t

            def emit_s5(e_t, va_t, o_t, gi):
                # step 4: [out | denom] = blockdiag(e).T @ [v | 1]
                for p in range(4):
                    op_t = o_ps.tile([100, VA_W], f32)
                    nc.tensor.matmul(
                        op_t[:], e_t[:, p, :], va_t[:, p, :],
                        start=True, stop=True,
                    )
                    rc_t = rcp.tile([100, 1], f32)
                    nc.vector.reciprocal(rc_t[:], op_t[:, D : D + 1])
                    if p % 2 == 0:
                        nc.scalar.activation(
                            o_t[:, p, :], op_t[:, 0:D], AF.Copy, scale=rc_t[:]
                        )
                    else:
                        nc.vector.tensor_scalar_mul(o_t[:, p, :], op_t[:, 0:D], rc_t[:])

            # Phase-major super-groups: batch each phase across SG groups
            # so the PE gets long dense matmul bursts and cross-engine
            # handoffs amortize.  Step 4 + the store trail by one
            # super-group and are emitted first so their Act/DVE divides
            # execute promptly and the output DMA issue doesn't stall the
            # gpsimd sequencer ahead of the next loads.
            # Queue-order design per iteration (engine programs execute in
            # emission order):
            #   loads(s+1)   — DMA issues a full iteration ahead of use;
            #                  qt+kt both on sync so the Act queue carries
            #                  no DMA work ahead of the divides
            #   s4(s-1,gi) interleaved with s1(s,gi) — the step1 matmuls
            #                  fill the PE while Act/DVE divides free the
            #                  o_ps PSUM bufs for the next step4 quad
            #   s2/s3(s)     — PE 32 MMs; exp/mult produce e for next iter
            loads = emit_loads(0)
            w0_t = singles.tile([128, 2, S], bf16)
            nc.sync.dma_start(w0_t[:], w0t_d.rearrange("(c p) s -> p c s", p=128))
            b_t = singles.tile([128, 2, 1], f32)
            nc.sync.dma_start(b_t[:], bias_d.rearrange("(c p) o -> p c o", p=128))
            def emit_s5_group(prev_s, e_t, va_t, gi):
                o_t = osb.tile([100, 4, D], bf16)
                emit_s5(e_t, va_t, o_t, gi)
                nc.gpsimd.dma_start(out_d[prev_s * SG + gi], o_t[:])

            prev = None
            for s in range(NSG):
                qts, kts, vas, mk_t = loads
                if s + 1 < NSG:
                    loads = emit_loads(s + 1)
                hts = []
                for gi in range(SG):
                    if prev is not None:
                        emit_s5_group(prev[0], prev[1][gi], prev[2][gi], gi)
                    hts.append(emit_s1(qts[gi], kts[gi]))
                e_ts = [emit_s3(hts[gi], mk_t, gi) for gi in range(SG)]
                prev = (s, e_ts, vas)
            for gi in range(SG):
                emit_s5_group(prev[0], prev[1][gi], prev[2][gi], gi)

    _split_multi_waits(nc)
    _cache["nc"] = nc
    return nc


def _make_in_maps(q, k, v, mask):
    in_maps = []
    for c in range(M):
        sl = slice(c * Bc, (c + 1) * Bc)
        # [NG, 128, 2, TOK]: each group's block contiguous in DRAM
        qs = np.ascontiguousarray(
            q[sl].reshape(NG, TOK, 2, 128).transpose(0, 3, 2, 1)
        ).astype(ml_dtypes.bfloat16)
        ks = np.ascontiguousarray(
            k[sl].reshape(NG, TOK, 2, 128).transpose(0, 3, 2, 1)
        ).astype(ml_dtypes.bfloat16)
        va = np.zeros((100, NG, 4, VA_W), dtype=ml_dtypes.bfloat16)
        va[:, :, :, :D] = v[sl].reshape(NG, 4, 100, D).transpose(2, 0, 1, 3).astype(ml_dtypes.bfloat16)
        va[:, :, :, D] = 1.0
        m5 = mask[sl].reshape(NG, 4, 2, 50, S).transpose(3, 0, 1, 2, 4)
        mk = np.zeros((100, NG, 4, 2, S), dtype=ml_dtypes.bfloat16)
        mk[0:50, :, :, 0, :] = m5[:, :, :, 0, :].astype(ml_dtypes.bfloat16)
        mk[50:100, :, :, 1, :] = m5[:, :, :, 1, :].astype(ml_dtypes.bfloat16)
        mk = mk.reshape(100, NSG, SG, 4, 2 * S)
        in_maps.append({"qt": qs, "kt": ks, "va": va, "mk": mk})
    return in_maps


def _run(q, k, v, mask, attn_w0, attn_w1, attn_w2, attn_bias, **run_kwargs):
    nc = _build()
    w1 = np.ascontiguousarray(attn_w1).astype(ml_dtypes.bfloat16)
    w2 = np.ascontiguousarray(attn_w2).astype(ml_dtypes.bfloat16)
    w0t = np.ascontiguousarray(
        np.asarray(attn_w0, dtype=np.float32).T
    ).astype(ml_dtypes.bfloat16)
    bias = np.ascontiguousarray(
        np.asarray(attn_bias, dtype=np.float32).reshape(D, 1)
    )
    in_maps = _make_in_maps(
        np.asarray(q, dtype=np.float32),
        np.asarray(k, dtype=np.float32),
        np.asarray(v, dtype=np.float32),
        np.asarray(mask),
    )
    for im in in_maps:
        im.update({"w1": w1, "w2": w2, "w0t": w0t, "bias": bias})
    res = run_bass_kernel_spmd(nc, in_maps, core_ids=list(range(M)), **run_kwargs)
    # out: [NG, 100, 4, D] -> batch = g*8 + j*2 + (p>=50), s = p%50
    out = np.concatenate(
        [
            r["out"].astype(np.float32)
            .reshape(NG, 2, 50, 4, D)
            .transpose(0, 3, 1, 2, 4)
            .reshape(Bc, S, D)
            for r in res.results
        ],
        axis=0,
    )
    return out, res


def kernel(q, k, v, mask, attn_w0, attn_w1, attn_w2, attn_bias):
    out, _ = _run(q, k, v, mask, attn_w0, attn_w1, attn_w2, attn_bias)
    return out
